# revision 5
# baseline (speedup 1.0000x reference)
"""Trainium2 Bass kernel for ItemEmbeddingLayer (embedding_lookup).

Reference computation:
    out = Q_matrix[items] @ skill_embedding[user]      # [8192, 128] f32

Sharding: items split 1024/core across 8 cores (data parallel); Q (bf16,
exact - Q is binary) and the single user's embedding row (bf16) replicated.

Per-core device kernel (computes out' = out^T; host transposes back):
  1. TWO dma_gather (SWDGE) ops with transpose=True, 512 rows each
     (elem_size=256 bf16 = 512B rows), landing directly in transposed
     layout qTr[128, 2, 512] with qTr[p, c, i] = Q[items[r*512+i], c*128+p].
     This replaces 8 indirect_dma_start + 16 PE transposes + 16 DVE copies
     of the previous version: SWDGE desc-gen is 994ns fixed + 0.34ns/desc.
     Two 512-idx gathers (not one 1024) because the transpose RX path needs
     2*n/16+2 descriptors per DMA ring and the SWDGE ring holds 128: n=1024
     deadlocks the DMA (observed: exec-unit unrecoverable), n<=896 is safe.
     The split also lets region-0 matmuls overlap gather 1's transfer.
  2. Matmuls with the embedding as stationary weights:
     ps[k, l] += emb[s,c,k]^T . qTr[s,c,l], both skill-chunks accumulated
     in fp32 PSUM, two 512-wide output regions (one PSUM bank each).
  3. DVE copies PSUM->SBUF as bf16, 2D DMA out per region (sync/scalar).
Host: concat per-core [128, 1024] -> [128, 8192] -> transpose -> [8192, 128].
"""

import numpy as np
import ml_dtypes

import concourse.bass as bass
import concourse.bacc as bacc
import concourse.mybir as mybir
from concourse.tile import TileContext
from concourse.bass_utils import run_bass_kernel_spmd

N_CORES = 8
L = 8192
LC = L // N_CORES          # 1024 items per core
S = 256
K = 128
R = 4096
P = 128
IW = LC // 16              # idx tile free dim (wrapped in 16 partitions)

# output regions (start_col, end_col); each must fit one PSUM bank (<=512 f32)
REGIONS = [(0, 512), (512, 1024)]


def build_bass() -> bass.Bass:
    nc = bacc.Bacc(trn_type="TRN2", dynamic_dma_scratch_size=131072)
    q = nc.declare_dram_parameter("q_bf16", [R, S], mybir.dt.bfloat16, isOutput=False)
    idx = nc.declare_dram_parameter("idx", [P, IW], mybir.dt.int16, isOutput=False)
    emb = nc.declare_dram_parameter("emb", [P, 2, K], mybir.dt.bfloat16, isOutput=False)
    out = nc.declare_dram_parameter("out", [K, LC], mybir.dt.bfloat16, isOutput=True)

    with (
        TileContext(nc) as tc,
        tc.tile_pool(name="main", bufs=1) as pool,
        tc.tile_pool(name="acc", bufs=1, space="PSUM") as apsum,
    ):
        idx_t = pool.tile([P, IW], mybir.dt.int16)
        nc.sync.dma_start(out=idx_t[:], in_=idx[:])
        emb_t = pool.tile([P, 2, K], mybir.dt.bfloat16)
        nc.scalar.dma_start(out=emb_t[:], in_=emb[:])

        qTs = []
        for r, (c0, c1) in enumerate(REGIONS):
            n = c1 - c0
            qT = pool.tile([P, 2, n], mybir.dt.bfloat16, tag=f"qT{r}")
            qTs.append(qT)
            nc.gpsimd.dma_gather(
                qT[:],
                q[:],
                idx_t[:, c0 // 16 : c1 // 16],
                n,        # num_idxs
                n,        # num_idxs_reg (immediate)
                S,        # elem_size (elements per gathered row)
                transpose=True,
            )

        engs = [nc.sync, nc.scalar]
        for r, (c0, c1) in enumerate(REGIONS):
            n = c1 - c0
            qT = qTs[r]
            ps = apsum.tile([P, n], mybir.dt.float32, tag=f"ps{r}")
            nc.tensor.matmul(ps[:], emb_t[:, 0, :], qT[:, 0, :], start=True, stop=False)
            nc.tensor.matmul(ps[:], emb_t[:, 1, :], qT[:, 1, :], start=False, stop=True)
            o = pool.tile([P, n], mybir.dt.bfloat16, tag=f"o{r}")
            nc.vector.tensor_copy(o[:], ps[:])
            engs[r % len(engs)].dma_start(out=out[:, c0:c1], in_=o[:])

    nc.compile()
    return nc


_CACHE: dict = {}


def get_nc() -> bass.Bass:
    if "nc" not in _CACHE:
        _CACHE["nc"] = build_bass()
    return _CACHE["nc"]


def make_in_maps(user, Q_matrix, items, skill_embedding):
    user = int(np.asarray(user))
    Q = np.asarray(Q_matrix, dtype=np.float32)
    items = np.asarray(items).astype(np.int64)
    E = np.ascontiguousarray(np.asarray(skill_embedding)[user], dtype=np.float32)
    q_bf = Q.astype(ml_dtypes.bfloat16)

    hi = E.astype(ml_dtypes.bfloat16)
    emb = np.empty((P, 2, K), dtype=ml_dtypes.bfloat16)
    for c in range(2):
        emb[:, c, :] = hi[c * P : (c + 1) * P, :]

    in_maps = []
    for i in range(N_CORES):
        it = items[i * LC : (i + 1) * LC].astype(np.int16)
        # [16, IW] block (element i at [i%16, i//16]), replicated across all
        # 8 Q7-cpu partition windows (ucode reads idxs from its own window)
        idx_arr = np.ascontiguousarray(np.tile(it.reshape(IW, 16).T, (8, 1)))
        in_maps.append({"q_bf16": q_bf, "idx": idx_arr, "emb": emb})
    return in_maps


def kernel(user, Q_matrix, items, skill_embedding, _trace=False, _result_box=None):
    in_maps = make_in_maps(user, Q_matrix, items, skill_embedding)
    res = run_bass_kernel_spmd(get_nc(), in_maps, list(range(N_CORES)), trace=_trace)
    if _result_box is not None:
        _result_box.append(res)
    full = np.concatenate(
        [np.asarray(res.results[i]["out"]).astype(np.float32) for i in range(N_CORES)],
        axis=1,
    )
    return np.ascontiguousarray(full.T, dtype=np.float32)
